# revision 12
# baseline (speedup 1.0000x reference)
"""Trainium2 Bass kernel for nn_AtomicPositionalEncoding.

kernel(**inputs): FULL x [256,1024,4] f32 -> FULL out [256,1024,128] f32.
Shards batch across 8 NeuronCores (32 examples each), one SPMD Bass program.

v4 layout ("scrambled out"): partition p owns 8 consecutive points of an
example (n = 8p + i), column j = 32jq + 8q + i (example b = 4jq + q).
The final expansion X[p | k, c, alpha] (alpha = (q,i), the point column)
keeps the POINT axis innermost so both tensor_tensor operands stream with
dense step-1 innermost APs -> DVE 2x perf mode.  The quad tile is DMA'd
to DRAM in that scrambled order ([jq, p, k, c, q, i], bf16) and the host
unscrambles + casts during the gather step.

Per-point tiles are k-major ([p, k, j]) so pgi/nm broadcast along middle
(free-stride-0) axes only.  masksC[p, jq, c, q, i] is built by a dense
bf16 is_equal against a host-supplied replicated-iota constant (2x mode).
Histogram scatter/gather (hist, scale, ohT, G) via PE as in v2.
"""

import os
import sys

import numpy as np

for p in ("/opt/trn_rl_repo", "/root/.axon_site/_ro/trn_rl_repo"):
    if os.path.isdir(p) and p not in sys.path:
        sys.path.insert(0, p)

import concourse.bass as bass
import concourse.bacc as bacc
import concourse.mybir as mybir
from concourse.tile import TileContext

F32 = mybir.dt.float32
BF16 = mybir.dt.bfloat16

EX = 32          # examples per core
NPT = 1024       # points per example
IPC = 8          # points per partition (i index)
NCOL = EX * IPC  # 256 point columns, j = 32jq + 8q + i
NQ = 8           # quads of 4 examples
C = 32
K = 4
D = 128
ETA = 4.0
RC = 6.0
Y00 = 0.5 / np.sqrt(np.pi)
C1 = np.sqrt(3.0 / (4.0 * np.pi))
RS = [0.0, 1.5, 3.0, 4.5]
SQD1 = float(np.sqrt(D - 1.0))

AF = mybir.ActivationFunctionType
OP = mybir.AluOpType

# groups of quads for hist/stats/final batching (pipeline granularity)
GROUPS = [[0], [1], [2, 3], [4, 5], [6, 7]]
# per-quad engine choice for the two final expansion passes ('v' or 'g')
S1Q = ['v', 'v', 'v', 'g', 'v', 'v', 'v', 'g']
S2Q = ['v', 'v', 'v', 'g', 'v', 'v', 'v', 'g']
# per-quad engine for the oh PSUM->SBUF copy ('v' or 'a')
OHE = ['a'] * 8
# env(r) ~= C4*(t^2 + A1*t + B1)*(t^2 + A2*t + B2), t = min(r^2, 36)
EA1, EB1 = -185.24528312900546, 14362.613501315343
EA2, EB2 = -71.91033462816625, 1292.888484103584
EC4 = 5.385167077448709e-08


def _consts_f32() -> np.ndarray:
    blockmask = np.zeros((128, 16), dtype=np.float32)                   # [128,16]
    for pp_ in range(128):
        for f in range(16):
            if pp_ // 32 == f // 4:
                blockmask[pp_, f] = 1.0
    ident = np.eye(128, dtype=np.float32)                               # [128,128]
    bconst = np.tile(np.array([np.pi / 2, 0.5, 1e-6 * np.sqrt(127.0), 0],
                          np.float32), (128, 1))
    # iotaRep[p, c, a] = c  (bf16 pattern stored as f32, cast on chip)
    iota_rep = np.tile(np.arange(C, dtype=np.float32)[None, :, None],
                       (128, 1, 32))                                    # [128,32,32]
    iota_d = np.tile(np.arange(C, dtype=np.float32), (128, 1))          # [128,32]
    return np.concatenate(
        [blockmask.ravel(), ident.ravel(), bconst.ravel(), iota_rep.ravel(),
         iota_d.ravel()]
    )


CF_SIZES = [128 * 16, 128 * 128, 128 * 4, 128 * C * 32, 128 * C]
CF_TOTAL = sum(CF_SIZES)


def build_nc() -> bass.Bass:
    nc = bacc.Bacc()
    x_d = nc.dram_tensor("x", [EX, NPT, 4], F32, kind="ExternalInput")
    cf_d = nc.dram_tensor("cf", [CF_TOTAL], F32, kind="ExternalInput")
    # scrambled: [jq, p, k, c, q, i] flattened per quad
    out_d = nc.dram_tensor("out", [NQ, 128, K * C * 32], BF16,
                           kind="ExternalOutput")

    with TileContext(nc) as tc:
        with (
            tc.tile_pool(name="persist", bufs=1) as pp,
            tc.tile_pool(name="xpool", bufs=8) as xp,
            tc.tile_pool(name="crpool", bufs=2) as cr,
            tc.tile_pool(name="ohpool", bufs=3) as bb,
            tc.tile_pool(name="outp", bufs=5) as op_,
            tc.tile_pool(name="ph", bufs=1, space="PSUM") as ph,     # histT
            tc.tile_pool(name="poh", bufs=2, space="PSUM") as poh,   # onehotT
            tc.tile_pool(name="pgb", bufs=2, space="PSUM") as pgb,   # G_B
            tc.tile_pool(name="pga", bufs=2, space="PSUM") as pga,   # G_A
            tc.tile_pool(name="psc", bufs=1, space="PSUM") as psc,   # scale
        ):
            ve, act, gp, pe, sy = nc.vector, nc.scalar, nc.gpsimd, nc.tensor, nc.sync

            # ---- constants ----
            offs = np.cumsum([0] + CF_SIZES)
            def cslice(i, shape):
                t = pp.tile(shape, F32, name=f"const{i}", tag=f"const{i}")
                src = cf_d[offs[i]:offs[i + 1]].rearrange("(p f) -> p f", p=shape[0])
                sy.dma_start(t, src)
                return t
            blockmask = cslice(0, [128, 16])
            identf = cslice(1, [128, 128])
            bconst = cslice(2, [128, 4])
            iota_rep_f = cslice(3, [128, C * 32])
            iota_d_f = cslice(4, [128, C])
            ident16 = pp.tile([128, 128], BF16, name="ident16", tag="ident16")
            ve.tensor_copy(ident16, identf)
            iota_rep = pp.tile([128, C * 32], BF16, name="iotarep", tag="iotarep")
            ve.tensor_copy(iota_rep, iota_rep_f)
            iota16 = pp.tile([128, C], BF16, name="iota16", tag="iota16")
            ve.tensor_copy(iota16, iota_d_f)

            # ---- x load: strided DMA into i-fold layout ----
            # x4[p, b, i, c] = x[b, 8p+i, c]
            x_sb = pp.tile([128, NCOL * 4], F32, name="x", tag="x")
            x4 = x_sb.rearrange("p (b i c) -> p b i c", b=EX, i=IPC)
            for h, eng in ((0, sy), (1, act)):
                dst = x4[:, 16 * h:16 * (h + 1)]
                src = x_d[16 * h:16 * (h + 1)].rearrange(
                    "b (p i) c -> p b i c", p=128)
                eng.dma_start(dst, src)
            clsf2 = x_sb.rearrange("p (j c) -> p j c", c=4)[:, :, 3:4] \
                        .rearrange("p j one -> p (j one)")  # [128,256] cls per col
            cls16 = pp.tile([128, NCOL], BF16, name="cls16", tag="cls16")
            ve.tensor_copy(cls16, clsf2)

            # ---- persistent per-point tensors ----
            def ptile(name, mult=1, dtype=F32):
                return pp.tile([128, NCOL * mult], dtype, name=name, tag=name)
            r = ptile("r")
            rsq = ptile("rsq")
            rinv = ptile("rinv")
            env = ptile("env")
            tmp3 = ptile("tmp3", 3)
            uC = ptile("uC", 3)            # [p, 3, j] c-major unit vecs
            difK = ptile("difK", K)        # [p, k, j]
            m4K = ptile("m4K", K)          # [p, k, j]
            prodmK = ptile("prodmK", K, BF16)   # [p, k, j]
            prodm2K = ptile("prodm2K", K, BF16)
            gK = ptile("gK", K, BF16)      # [p, k, j]
            pgK = ptile("pgK", K, BF16)
            pg2K = ptile("pg2K", K, BF16)
            pgiK = ptile("pgiK", K, BF16)
            ssum = ptile("ssum", 1, BF16)  # S = sum_k pg
            msum = ptile("msum", 1, BF16)  # msq = sum_k pg^2
            var = ptile("var")
            std = ptile("std")
            istd = ptile("istd")
            nm16 = ptile("nm16", 1, BF16)  # negmistd, bf16
            # masksC[p, jq, c, q, i] bf16 (alpha-inner, for the final STT)
            masksC = pp.tile([128, NQ * C * 32], BF16, name="masksC",
                             tag="masksC")
            # masksJ[p, jq, i, q, c] bf16 (c-inner, for hist/ohT PE ops)
            masksJ = pp.tile([128, NQ * C * 32], BF16, name="masksJ",
                             tag="masksJ")
            # prodm25[p, jq, i, q, k] bf16 (PE weights layout)
            prodm25 = pp.tile([128, NQ * IPC * 4 * K], BF16, name="prodm25",
                              tag="prodm25")
            # cls permuted to (jq, i, q) column order for the masksJ STT
            cls_perm = pp.tile([128, NCOL], BF16, name="cls_perm",
                               tag="cls_perm")
            # 32 zero pad columns so every quad has a 32-wide weight window
            squad = pp.tile([128, 160], BF16, name="squad", tag="squad")
            ve.memset(squad, 0.0)

            masksC4 = masksC.rearrange("p (jq c q i) -> p jq c q i",
                                       jq=NQ, c=C, q=4)
            masksJ5 = masksJ.rearrange("p (jq i q c) -> p jq i q c",
                                       jq=NQ, i=IPC, q=4)
            masksJ3 = masksJ.rearrange("p (jj c) -> p jj c", c=C)
            prodm255 = prodm25.rearrange("p (jq i q k) -> p jq i q k",
                                         jq=NQ, i=IPC, q=4)
            kj = lambda t: t.rearrange("p (k j) -> p k j", k=K)
            uC3 = uC.rearrange("p (c j) -> p c j", c=3)

            # ---- stage 1: per-point geometry -> prodmK (k-major) ----
            def stage1(c0, c1):
                J = c1 - c0
                xyzC = x4.rearrange("p b i c -> p (b i) c")[:, c0:c1, 0:3] \
                         .rearrange("p j c -> p c j")        # [p, 3, J] view
                t3 = tmp3.rearrange("p (c j) -> p c j", c=3)
                sq = t3[:, :, c0:c1]
                gp.tensor_tensor(sq, xyzC, xyzC, OP.mult)
                rv = rsq[:, c0:c1].unsqueeze(1)
                gp.tensor_tensor(rv, sq[:, 0:1, :], sq[:, 1:2, :], OP.add)
                gp.tensor_tensor(rv, rv, sq[:, 2:3, :], OP.add)
                act.activation(r[:, c0:c1], rsq[:, c0:c1], AF.Sqrt)
                ve.reciprocal_approx_fast(rinv[:, c0:c1], r[:, c0:c1])
                # C1 folded into rinv (only consumer is u*C1*m below)
                ve.tensor_scalar(out=rinv[:, c0:c1], in0=rinv[:, c0:c1],
                                 scalar1=float(C1), scalar2=None, op0=OP.mult)
                # env via even quartic in t = min(r^2, 36); C4 folded into g1
                tv = env[:, c0:c1]
                ve.tensor_scalar(out=tv, in0=rsq[:, c0:c1],
                                 scalar1=36.0, scalar2=None, op0=OP.min)
                t2 = t3[:, 0, c0:c1]
                g1 = t3[:, 1, c0:c1]
                g2 = t3[:, 2, c0:c1]
                gp.tensor_tensor(t2, tv, tv, OP.mult)
                ve.scalar_tensor_tensor(out=g1, in0=tv, scalar=float(EA1),
                                        in1=t2, op0=OP.mult, op1=OP.add)
                ve.tensor_scalar(out=g1, in0=g1, scalar1=float(EB1),
                                 scalar2=float(EC4), op0=OP.add, op1=OP.mult)
                ve.scalar_tensor_tensor(out=g2, in0=tv, scalar=float(EA2),
                                        in1=t2, op0=OP.mult, op1=OP.add)
                ve.tensor_scalar(out=g2, in0=g2, scalar1=float(EB2),
                                 scalar2=None, op0=OP.add)
                gp.tensor_tensor(tv, g1, g2, OP.mult)
                # radial (k-major): dif = r - Rs[k]; rad = exp(-eta*dif^2)
                dv = kj(difK)[:, :, c0:c1]
                r_b = r[:, c0:c1].unsqueeze(1).broadcast_to([128, K, J])
                for kk in range(K):
                    ve.tensor_scalar(out=dv[:, kk], in0=r_b[:, kk],
                                     scalar1=float(RS[kk]), scalar2=None,
                                     op0=OP.subtract)
                act.activation(dv, dv, AF.Square, scale=float(np.sqrt(ETA)))
                act.activation(dv, dv, AF.Exp, scale=-1.0)
                # m4K = radial * env
                mv = kj(m4K)[:, :, c0:c1]
                env_b = env[:, c0:c1].unsqueeze(1).broadcast_to([128, K, J])
                gp.tensor_tensor(mv, dv, env_b, OP.mult)
                # uC = xyz * (C1*rinv) (c-major)
                uv = uC3[:, :, c0:c1]
                rinv_b = rinv[:, c0:c1].unsqueeze(1).broadcast_to([128, 3, J])
                gp.tensor_tensor(uv, xyzC, rinv_b, OP.mult)
                # prodmK: k=0 -> Y00*m0 ; k=1,2,3 -> (C1*u)[y,z,x]*m[k]
                pv = kj(prodmK)[:, :, c0:c1]
                act.mul(pv[:, 0], mv[:, 0], float(Y00))
                for kk, cc in ((1, 1), (2, 2), (3, 0)):
                    gp.tensor_tensor(pv[:, kk], uv[:, cc], mv[:, kk], OP.mult)

            stage1(0, 64)       # quads 0-1 express
            stage1(64, NCOL)    # the rest

            # ---- per-group machinery ----
            def do_group(grp):
                L = len(grp)
                jq0 = grp[0]
                c0, c1 = 32 * jq0, 32 * (jq0 + L)
                # masksC via dense bf16 is_equal (2x mode)
                cls_b = cls16.rearrange("p (jq a) -> p jq a", jq=NQ)[:, jq0:jq0 + L] \
                    .unsqueeze(2).broadcast_to([128, L, C, 32])
                iota_b = iota_rep.rearrange("p (c a) -> p c a", c=C) \
                    .unsqueeze(1).broadcast_to([128, L, C, 32])
                ve.tensor_tensor(
                    masksC.rearrange("p (jq ca) -> p jq ca", jq=NQ)[:, jq0:jq0 + L]
                        .rearrange("p jq (c a) -> p jq c a", c=C),
                    cls_b, iota_b, OP.is_equal)
                # cls_perm chunk: (jq, i, q) order for the masksJ STT
                ve.tensor_copy(
                    cls_perm.rearrange("p (jq i q) -> p jq i q",
                                       jq=NQ, i=IPC)[:, jq0:jq0 + L],
                    cls16.rearrange("p (jq q i) -> p jq i q",
                                    jq=NQ, q=4)[:, jq0:jq0 + L])
                # masksJ (c-inner): ACT replicates cls over c (any-AP copy),
                # then a dense bf16 is_equal runs at 2x on DVE
                clsp_b = cls_perm[:, c0:c1].unsqueeze(2) \
                    .broadcast_to([128, 32 * L, C])
                cls_rep = cr.tile([128, 32 * L * C], BF16, name="cls_rep",
                                  tag="cls_rep")
                crv = cls_rep.rearrange("p (jj c) -> p jj c", c=C)
                act.copy(crv, clsp_b)
                iotaj_b = iota16.unsqueeze(1).broadcast_to([128, 32 * L, C])
                ve.tensor_tensor(masksJ3[:, 32 * jq0:32 * (jq0 + L), :],
                                 crv, iotaj_b, OP.is_equal)
                # prodm2K = prodmK^2 (dense 2x), then per-quad PE-layout copies
                ve.tensor_tensor(kj(prodm2K)[:, :, c0:c1], kj(prodmK)[:, :, c0:c1],
                                 kj(prodmK)[:, :, c0:c1], OP.mult)
                for jq in grp:
                    ve.tensor_copy(
                        prodm255[:, jq],
                        kj(prodm2K)[:, :, 32 * jq:32 * (jq + 1)]
                            .rearrange("p k (q i) -> p i q k", q=4))
                # histT per quad: [16=(q,k), 128L], weights=prodm2, moving=masks
                hist_ps = ph.tile([16, 128 * L], F32, name="hist_ps", tag="hist_ps")
                for jx, jq in enumerate(grp):
                    for i in range(IPC):
                        lhsT = prodm255[:, jq, i].rearrange("p q k -> p (q k)")
                        rhs = masksJ5[:, jq, i].rearrange("p q c -> p (q c)")
                        pe.matmul(hist_ps[:, 128 * jx:128 * (jx + 1)], lhsT, rhs,
                                  start=(i == 0), stop=(i == IPC - 1))
                # sqrt(hist) -> transpose -> 1/x on the narrow side
                # squad = min(1/sqrt(hist), 1e12) * blockmask
                scaleT = xp.tile([16, 128 * L], F32, name="scaleT", tag="scaleT")
                act.activation(scaleT, hist_ps, AF.Sqrt)
                scale_ps = psc.tile([128, 16 * L], F32, name="scale_ps", tag="scale_ps")
                for jx in range(L):
                    pe.transpose(scale_ps[:, 16 * jx:16 * (jx + 1)],
                                 scaleT[:, 128 * jx:128 * (jx + 1)],
                                 identf[:16, :16])
                scinv = xp.tile([128, 16 * L], F32, name="scinv", tag="scinv")
                ve.reciprocal(scinv, scale_ps)
                sq_view = squad[:, 0:128].rearrange("p (jq f) -> p jq f", f=16)
                bm_b = blockmask.unsqueeze(1).broadcast_to([128, L, 16])
                ve.scalar_tensor_tensor(
                    out=sq_view[:, jq0:jq0 + L],
                    in0=scinv.rearrange("p (l f) -> p l f", f=16),
                    scalar=1e12, in1=bm_b, op0=OP.min, op1=OP.mult)

                # per pair of quads: onehotT, G_B (partition-packed), G_A
                pairs = [grp[i:i + 2] for i in range(0, L, 2)]
                for pair in pairs:
                    ohs = []
                    for jq in pair:
                        oh_ps = poh.tile([128, NPT], BF16, name="oh_ps",
                                         tag="oh_ps")
                        for i in range(IPC):
                            lhsT = masksJ5[:, jq, i].rearrange("p q c -> p (q c)")
                            pe.transpose(oh_ps[:, 128 * i:128 * (i + 1)],
                                         lhsT, ident16)
                        oh_sb = bb.tile([128, NPT], BF16, name="oh_sb",
                                        tag="oh_sb")
                        if OHE[jq] == 'v':
                            ve.tensor_copy(oh_sb, oh_ps)
                        else:
                            act.copy(oh_sb, oh_ps)
                        ohs.append(oh_sb)
                    # 32-col squad windows keep the packed PSUM fully written
                    P = 32 * len(pair)
                    gb_sb = xp.tile([P, NPT], BF16, name="gb_sb", tag="gb_sb")
                    for h in range(2):
                        gb_ps = pgb.tile([P, 512], F32, name="gb_ps", tag="gb_ps")
                        for px, jq in enumerate(pair):
                            pe.matmul(gb_ps[32 * px:32 * px + 32, :],
                                      squad[:, 16 * jq:16 * jq + 32],
                                      ohs[px][:, 512 * h:512 * (h + 1)],
                                      start=True, stop=True)
                        act.copy(gb_sb[:, 512 * h:512 * (h + 1)], gb_ps)
                    for px, jq in enumerate(pair):
                        ga_ps = pga.tile([128, 128], BF16, name="ga_ps",
                                         tag="ga_ps")
                        p0 = 32 * px
                        for i in range(IPC):
                            pe.transpose(
                                ga_ps[:, 16 * i:16 * (i + 1)],
                                gb_sb[p0:p0 + 16, 128 * i:128 * (i + 1)],
                                ident16[p0:p0 + 16, p0:p0 + 16])
                        # gK[p, k, (jq q i)] <- ga_ps[p, (i q k)]
                        ve.tensor_copy(
                            gK.rearrange("p (k jq q i) -> p jq k q i",
                                         jq=NQ, q=4, i=IPC)[:, jq],
                            ga_ps.rearrange("p (i q k) -> p k q i", i=IPC, q=4))

                # stats over this group's columns (k-major)
                cs = slice(c0, c1)
                pgv = kj(pgK)[:, :, cs]
                ve.tensor_tensor(pgv, kj(prodmK)[:, :, cs], kj(gK)[:, :, cs],
                                 OP.mult)
                # S = sum_k pg ; msq = sum_k pg^2  (slice adds, dense 2x)
                ve.tensor_tensor(kj(pg2K)[:, :, cs], pgv, pgv, OP.mult)
                ve.tensor_tensor(ssum[:, cs].unsqueeze(1), pgv[:, 0:1],
                                 pgv[:, 1:2], OP.add)
                ve.tensor_tensor(msum[:, cs].unsqueeze(1), kj(pg2K)[:, 0:1, cs],
                                 kj(pg2K)[:, 1:2, cs], OP.add)
                for kk in (2, 3):
                    ve.tensor_tensor(ssum[:, cs].unsqueeze(1),
                                     ssum[:, cs].unsqueeze(1),
                                     pgv[:, kk:kk + 1], OP.add)
                    ve.tensor_tensor(msum[:, cs].unsqueeze(1),
                                     msum[:, cs].unsqueeze(1),
                                     kj(pg2K)[:, kk:kk + 1, cs], OP.add)
                # var*(D-1) = msq - S^2/D ; istd = 1/(sqrt(var)+eps')
                ve.scalar_tensor_tensor(out=var[:, cs], in0=ssum[:, cs],
                                        scalar=float(-1.0 / D), in1=ssum[:, cs],
                                        op0=OP.mult, op1=OP.mult)
                ve.scalar_tensor_tensor(out=var[:, cs], in0=var[:, cs], scalar=0.0,
                                        in1=msum[:, cs], op0=OP.add, op1=OP.add)
                ve.tensor_scalar(out=var[:, cs], in0=var[:, cs], scalar1=0.0,
                                 scalar2=None, op0=OP.max)
                act.activation(std[:, cs], var[:, cs], AF.Sqrt)
                act.activation(std[:, cs], std[:, cs], AF.Identity,
                               bias=bconst[:, 2:3], scale=1.0)
                ve.reciprocal_approx_fast(istd[:, cs], std[:, cs])
                ve.scalar_tensor_tensor(out=nm16[:, cs], in0=ssum[:, cs],
                                        scalar=float(-SQD1 / D), in1=istd[:, cs],
                                        op0=OP.mult, op1=OP.mult)
                # pgi = (pg * SQD1) * istd  (k-major bf16)
                istd_b = istd[:, cs].unsqueeze(1).broadcast_to([128, K, c1 - c0])
                ve.scalar_tensor_tensor(out=kj(pgiK)[:, :, cs], in0=pgv,
                                        scalar=float(SQD1), in1=istd_b,
                                        op0=OP.mult, op1=OP.mult)

            def final(grp):
                for jq in grp:
                    js = slice(32 * jq, 32 * (jq + 1))
                    X = op_.tile([128, K * C * 32], BF16, name="X", tag="X")
                    X4 = X.rearrange("p (k c a) -> p k c a", k=K, c=C)
                    mask_b = masksC4[:, jq].rearrange("p c q i -> p (c q i)") \
                        .rearrange("p (c a) -> p c a", c=C) \
                        .unsqueeze(1).broadcast_to([128, K, C, 32])
                    pgi_b = kj(pgiK)[:, :, js].unsqueeze(2) \
                        .broadcast_to([128, K, C, 32])
                    e1 = ve if S1Q[jq] == 'v' else gp
                    e1.tensor_tensor(X4, mask_b, pgi_b, OP.mult)
                    nm_b = nm16[:, js].unsqueeze(1).unsqueeze(1) \
                        .broadcast_to([128, K, C, 32])
                    e2 = ve if S2Q[jq] == 'v' else gp
                    e2.tensor_tensor(X4, X4, nm_b, OP.add)
                    sy.dma_start(out_d[jq], X)

            do_group(GROUPS[0])
            do_group(GROUPS[1])
            final(GROUPS[0])
            do_group(GROUPS[2])
            final(GROUPS[1])
            do_group(GROUPS[3])
            final(GROUPS[2])
            do_group(GROUPS[4])
            final(GROUPS[3])
            final(GROUPS[4])

    if not nc.is_finalized():
        nc.finalize()
    return nc


_NC = None


def _get_nc():
    global _NC
    if _NC is None:
        _NC = build_nc()
    return _NC


def _unscramble(raw: np.ndarray) -> np.ndarray:
    """[NQ, 128, K*C*32] scrambled -> [EX, NPT, D] (still source dtype)."""
    t = np.asarray(raw).reshape(NQ, 128, K, C, 4, IPC)
    # [jq, p, k, c, q, i] -> [jq, q, p, i, k, c]
    return np.transpose(t, (0, 4, 1, 5, 2, 3)).reshape(EX, NPT, D)


def kernel(x: np.ndarray) -> np.ndarray:
    from concourse.bass_utils import run_bass_kernel_spmd

    x = np.ascontiguousarray(np.asarray(x, dtype=np.float32))
    B = x.shape[0]
    n_cores = 8
    per = B // n_cores
    cf = _consts_f32()
    nc = _get_nc()
    in_maps = [
        {"x": x[i * per:(i + 1) * per], "cf": cf} for i in range(n_cores)
    ]
    res = run_bass_kernel_spmd(nc, in_maps, core_ids=list(range(n_cores)))
    return np.concatenate(
        [_unscramble(r["out"]).astype(np.float32) for r in res.results], axis=0)


if __name__ == "__main__":
    from concourse.bass_interp import CoreSim

    rng = np.random.default_rng(0)
    x = (rng.standard_normal((EX, NPT, 4)) * 2.0).astype(np.float32)
    x[..., 3] = rng.integers(0, C, size=(EX, NPT)).astype(np.float32)
    nc = build_nc()
    sim = CoreSim(nc)
    sim.tensor("x")[:] = x
    sim.tensor("cf")[:] = _consts_f32()
    sim.simulate()
    got = _unscramble(np.array(sim.tensor("out"))).astype(np.float32)

    xyz = x[..., :3]; clsf_ = x[..., 3]
    r = np.sqrt((xyz * xyz).sum(-1)); rinv = 1.0 / r
    radial = np.exp(-ETA * (np.array(RS, np.float32)[None, None] - r[..., None]) ** 2)
    env = 0.5 * np.cos(np.pi * np.minimum(r, RC) / RC) + 0.5
    sh = np.stack([np.full_like(r, Y00), C1 * xyz[..., 1] * rinv,
                   C1 * xyz[..., 2] * rinv, C1 * xyz[..., 0] * rinv], -1)
    prod = sh * radial * env[..., None]
    onehot = (clsf_[..., None] == np.arange(C, dtype=np.float32)).astype(np.float32)
    pos = (prod[..., :, None] * onehot[..., None, :]).reshape(EX, NPT, D)
    norm = np.sqrt((pos * pos).sum(1, keepdims=True))
    pos = pos / np.maximum(norm, 1e-12)
    mean_ = pos.mean(-1, keepdims=True)
    std_ = pos.std(-1, ddof=1, keepdims=True)
    want = (pos - mean_) / (std_ + 1e-6)
    err = np.abs(got - want)
    rel = np.linalg.norm((got - want).ravel()) / np.linalg.norm(want.ravel())
    print("sim absmax err:", err.max(), "rel:", rel, "ref absmax:", np.abs(want).max())


# revision 13
# speedup vs baseline: 1.4923x; 1.4923x over previous
"""Trainium2 Bass kernel for nn_AtomicPositionalEncoding.

kernel(**inputs): FULL x [256,1024,4] f32 -> FULL out [256,1024,128] f32.
Shards batch across 8 NeuronCores (32 examples each), one SPMD Bass program.

v4 layout ("scrambled out"): partition p owns 8 consecutive points of an
example (n = 8p + i), column j = 32jq + 8q + i (example b = 4jq + q).
The final expansion X[p | k, c, alpha] (alpha = (q,i), the point column)
keeps the POINT axis innermost so both tensor_tensor operands stream with
dense step-1 innermost APs -> DVE 2x perf mode.  The quad tile is DMA'd
to DRAM in that scrambled order ([jq, p, k, c, q, i], bf16) and the host
unscrambles + casts during the gather step.

Per-point tiles are k-major ([p, k, j]) so pgi/nm broadcast along middle
(free-stride-0) axes only.  masksC[p, jq, c, q, i] is built by a dense
bf16 is_equal against a host-supplied replicated-iota constant (2x mode).
Histogram scatter/gather (hist, scale, ohT, G) via PE as in v2.
"""

import os
import sys

import numpy as np

for p in ("/opt/trn_rl_repo", "/root/.axon_site/_ro/trn_rl_repo"):
    if os.path.isdir(p) and p not in sys.path:
        sys.path.insert(0, p)

import concourse.bass as bass
import concourse.bacc as bacc
import concourse.mybir as mybir
from concourse.tile import TileContext

F32 = mybir.dt.float32
BF16 = mybir.dt.bfloat16

EX = 32          # examples per core
NPT = 1024       # points per example
IPC = 8          # points per partition (i index)
NCOL = EX * IPC  # 256 point columns, j = 32jq + 8q + i
NQ = 8           # quads of 4 examples
C = 32
K = 4
D = 128
ETA = 4.0
RC = 6.0
Y00 = 0.5 / np.sqrt(np.pi)
C1 = np.sqrt(3.0 / (4.0 * np.pi))
RS = [0.0, 1.5, 3.0, 4.5]
SQD1 = float(np.sqrt(D - 1.0))

AF = mybir.ActivationFunctionType
OP = mybir.AluOpType

# groups of quads for hist/stats/final batching (pipeline granularity)
GROUPS = [[0], [1], [2, 3], [4, 5, 6, 7]]
# per-quad engine choice for the two final expansion passes ('v' or 'g')
S1Q = ['v'] * 8
S2Q = ['v'] * 8
# per-quad engine for the oh PSUM->SBUF copy ('v' or 'a')
OHE = ['a'] * 8
# env(r) ~= C4*(t^2 + A1*t + B1)*(t^2 + A2*t + B2), t = min(r^2, 36)
EA1, EB1 = -185.24528312900546, 14362.613501315343
EA2, EB2 = -71.91033462816625, 1292.888484103584
EC4 = 5.385167077448709e-08


def _consts_f32() -> np.ndarray:
    blockmask = np.zeros((128, 16), dtype=np.float32)                   # [128,16]
    for pp_ in range(128):
        for f in range(16):
            if pp_ // 32 == f // 4:
                blockmask[pp_, f] = 1.0
    ident = np.eye(128, dtype=np.float32)                               # [128,128]
    bconst = np.tile(np.array([np.pi / 2, 0.5, 1e-6 * np.sqrt(127.0), 0],
                          np.float32), (128, 1))
    # iotaRep[p, c, a] = c  (bf16 pattern stored as f32, cast on chip)
    iota_rep = np.tile(np.arange(C, dtype=np.float32)[None, :, None],
                       (128, 1, 32))                                    # [128,32,32]
    iota_d = np.tile(np.arange(C, dtype=np.float32), (128, 1))          # [128,32]
    return np.concatenate(
        [blockmask.ravel(), ident.ravel(), bconst.ravel(), iota_rep.ravel(),
         iota_d.ravel()]
    )


CF_SIZES = [128 * 16, 128 * 128, 128 * 4, 128 * C * 32, 128 * C]
CF_TOTAL = sum(CF_SIZES)


def build_nc() -> bass.Bass:
    nc = bacc.Bacc()
    x_d = nc.dram_tensor("x", [EX, NPT, 4], F32, kind="ExternalInput")
    cf_d = nc.dram_tensor("cf", [CF_TOTAL], F32, kind="ExternalInput")
    # scrambled: [jq, p, k, c, q, i] flattened per quad
    out_d = nc.dram_tensor("out", [NQ, 128, K * C * 32], BF16,
                           kind="ExternalOutput")

    with TileContext(nc) as tc:
        with (
            tc.tile_pool(name="persist", bufs=1) as pp,
            tc.tile_pool(name="xpool", bufs=8) as xp,
            tc.tile_pool(name="crpool", bufs=2) as cr,
            tc.tile_pool(name="ohpool", bufs=3) as bb,
            tc.tile_pool(name="outp", bufs=5) as op_,
            tc.tile_pool(name="ph", bufs=1, space="PSUM") as ph,     # histT
            tc.tile_pool(name="poh", bufs=2, space="PSUM") as poh,   # onehotT
            tc.tile_pool(name="pgb", bufs=2, space="PSUM") as pgb,   # G_B
            tc.tile_pool(name="pga", bufs=2, space="PSUM") as pga,   # G_A
            tc.tile_pool(name="psc", bufs=1, space="PSUM") as psc,   # scale
        ):
            ve, act, gp, pe, sy = nc.vector, nc.scalar, nc.gpsimd, nc.tensor, nc.sync

            # ---- constants ----
            offs = np.cumsum([0] + CF_SIZES)
            def cslice(i, shape):
                t = pp.tile(shape, F32, name=f"const{i}", tag=f"const{i}")
                src = cf_d[offs[i]:offs[i + 1]].rearrange("(p f) -> p f", p=shape[0])
                sy.dma_start(t, src)
                return t
            blockmask = cslice(0, [128, 16])
            identf = cslice(1, [128, 128])
            bconst = cslice(2, [128, 4])
            iota_rep_f = cslice(3, [128, C * 32])
            iota_d_f = cslice(4, [128, C])
            ident16 = pp.tile([128, 128], BF16, name="ident16", tag="ident16")
            ve.tensor_copy(ident16, identf)
            iota_rep = pp.tile([128, C * 32], BF16, name="iotarep", tag="iotarep")
            ve.tensor_copy(iota_rep, iota_rep_f)
            iota16 = pp.tile([128, C], BF16, name="iota16", tag="iota16")
            ve.tensor_copy(iota16, iota_d_f)

            # ---- x load: strided DMA into i-fold layout ----
            # x4[p, b, i, c] = x[b, 8p+i, c]
            x_sb = pp.tile([128, NCOL * 4], F32, name="x", tag="x")
            x4 = x_sb.rearrange("p (b i c) -> p b i c", b=EX, i=IPC)
            for h, eng in ((0, sy), (1, act)):
                dst = x4[:, 16 * h:16 * (h + 1)]
                src = x_d[16 * h:16 * (h + 1)].rearrange(
                    "b (p i) c -> p b i c", p=128)
                eng.dma_start(dst, src)
            clsf2 = x_sb.rearrange("p (j c) -> p j c", c=4)[:, :, 3:4] \
                        .rearrange("p j one -> p (j one)")  # [128,256] cls per col
            cls16 = pp.tile([128, NCOL], BF16, name="cls16", tag="cls16")
            ve.tensor_copy(cls16, clsf2)

            # ---- persistent per-point tensors ----
            def ptile(name, mult=1, dtype=F32):
                return pp.tile([128, NCOL * mult], dtype, name=name, tag=name)
            r = ptile("r")
            rsq = ptile("rsq")
            rinv = ptile("rinv")
            env = ptile("env")
            tmp3 = ptile("tmp3", 3)
            uC = ptile("uC", 3)            # [p, 3, j] c-major unit vecs
            difK = ptile("difK", K)        # [p, k, j]
            m4K = ptile("m4K", K)          # [p, k, j]
            prodmK = ptile("prodmK", K, BF16)   # [p, k, j]
            prodm2K = ptile("prodm2K", K, BF16)
            gK = ptile("gK", K, BF16)      # [p, k, j]
            pgK = ptile("pgK", K, BF16)
            pg2K = ptile("pg2K", K, BF16)
            pgiK = ptile("pgiK", K, BF16)
            ssum = ptile("ssum", 1, BF16)  # S = sum_k pg
            msum = ptile("msum", 1, BF16)  # msq = sum_k pg^2
            var = ptile("var")
            std = ptile("std")
            istd = ptile("istd")
            nm16 = ptile("nm16", 1, BF16)  # negmistd, bf16
            # masksC[p, jq, c, q, i] bf16 (alpha-inner, for the final STT)
            masksC = pp.tile([128, NQ * C * 32], BF16, name="masksC",
                             tag="masksC")
            # masksJ[p, jq, i, q, c] bf16 (c-inner, for hist/ohT PE ops)
            masksJ = pp.tile([128, NQ * C * 32], BF16, name="masksJ",
                             tag="masksJ")
            # prodm25[p, jq, i, q, k] bf16 (PE weights layout)
            prodm25 = pp.tile([128, NQ * IPC * 4 * K], BF16, name="prodm25",
                              tag="prodm25")
            # cls permuted to (jq, i, q) column order for the masksJ STT
            cls_perm = pp.tile([128, NCOL], BF16, name="cls_perm",
                               tag="cls_perm")
            # 32 zero pad columns so every quad has a 32-wide weight window
            squad = pp.tile([128, 160], BF16, name="squad", tag="squad")
            ve.memset(squad, 0.0)

            masksC4 = masksC.rearrange("p (jq c q i) -> p jq c q i",
                                       jq=NQ, c=C, q=4)
            masksJ5 = masksJ.rearrange("p (jq i q c) -> p jq i q c",
                                       jq=NQ, i=IPC, q=4)
            masksJ3 = masksJ.rearrange("p (jj c) -> p jj c", c=C)
            prodm255 = prodm25.rearrange("p (jq i q k) -> p jq i q k",
                                         jq=NQ, i=IPC, q=4)
            kj = lambda t: t.rearrange("p (k j) -> p k j", k=K)
            uC3 = uC.rearrange("p (c j) -> p c j", c=3)

            # ---- stage 1: per-point geometry -> prodmK (k-major) ----
            def stage1(c0, c1):
                J = c1 - c0
                xyzC = x4.rearrange("p b i c -> p (b i) c")[:, c0:c1, 0:3] \
                         .rearrange("p j c -> p c j")        # [p, 3, J] view
                t3 = tmp3.rearrange("p (c j) -> p c j", c=3)
                sq = t3[:, :, c0:c1]
                gp.tensor_tensor(sq, xyzC, xyzC, OP.mult)
                rv = rsq[:, c0:c1].unsqueeze(1)
                gp.tensor_tensor(rv, sq[:, 0:1, :], sq[:, 1:2, :], OP.add)
                gp.tensor_tensor(rv, rv, sq[:, 2:3, :], OP.add)
                act.activation(r[:, c0:c1], rsq[:, c0:c1], AF.Sqrt)
                ve.reciprocal_approx_fast(rinv[:, c0:c1], r[:, c0:c1])
                # C1 folded into rinv (only consumer is u*C1*m below)
                ve.tensor_scalar(out=rinv[:, c0:c1], in0=rinv[:, c0:c1],
                                 scalar1=float(C1), scalar2=None, op0=OP.mult)
                # env via even quartic in t = min(r^2, 36); C4 folded into g1
                tv = env[:, c0:c1]
                ve.tensor_scalar(out=tv, in0=rsq[:, c0:c1],
                                 scalar1=36.0, scalar2=None, op0=OP.min)
                t2 = t3[:, 0, c0:c1]
                g1 = t3[:, 1, c0:c1]
                g2 = t3[:, 2, c0:c1]
                gp.tensor_tensor(t2, tv, tv, OP.mult)
                ve.scalar_tensor_tensor(out=g1, in0=tv, scalar=float(EA1),
                                        in1=t2, op0=OP.mult, op1=OP.add)
                ve.tensor_scalar(out=g1, in0=g1, scalar1=float(EB1),
                                 scalar2=float(EC4), op0=OP.add, op1=OP.mult)
                ve.scalar_tensor_tensor(out=g2, in0=tv, scalar=float(EA2),
                                        in1=t2, op0=OP.mult, op1=OP.add)
                ve.tensor_scalar(out=g2, in0=g2, scalar1=float(EB2),
                                 scalar2=None, op0=OP.add)
                gp.tensor_tensor(tv, g1, g2, OP.mult)
                # radial (k-major): dif = r - Rs[k]; rad = exp(-eta*dif^2)
                dv = kj(difK)[:, :, c0:c1]
                r_b = r[:, c0:c1].unsqueeze(1).broadcast_to([128, K, J])
                for kk in range(K):
                    ve.tensor_scalar(out=dv[:, kk], in0=r_b[:, kk],
                                     scalar1=float(RS[kk]), scalar2=None,
                                     op0=OP.subtract)
                act.activation(dv, dv, AF.Square, scale=float(np.sqrt(ETA)))
                act.activation(dv, dv, AF.Exp, scale=-1.0)
                # m4K = radial * env
                mv = kj(m4K)[:, :, c0:c1]
                env_b = env[:, c0:c1].unsqueeze(1).broadcast_to([128, K, J])
                gp.tensor_tensor(mv, dv, env_b, OP.mult)
                # uC = xyz * (C1*rinv) (c-major)
                uv = uC3[:, :, c0:c1]
                rinv_b = rinv[:, c0:c1].unsqueeze(1).broadcast_to([128, 3, J])
                gp.tensor_tensor(uv, xyzC, rinv_b, OP.mult)
                # prodmK: k=0 -> Y00*m0 ; k=1,2,3 -> (C1*u)[y,z,x]*m[k]
                pv = kj(prodmK)[:, :, c0:c1]
                act.mul(pv[:, 0], mv[:, 0], float(Y00))
                for kk, cc in ((1, 1), (2, 2), (3, 0)):
                    gp.tensor_tensor(pv[:, kk], uv[:, cc], mv[:, kk], OP.mult)

            stage1(0, 64)       # quads 0-1 express
            stage1(64, NCOL)    # the rest

            # ---- per-group machinery ----
            def do_group(grp):
                L = len(grp)
                jq0 = grp[0]
                c0, c1 = 32 * jq0, 32 * (jq0 + L)
                # masksC via dense bf16 is_equal (2x mode)
                cls_b = cls16.rearrange("p (jq a) -> p jq a", jq=NQ)[:, jq0:jq0 + L] \
                    .unsqueeze(2).broadcast_to([128, L, C, 32])
                iota_b = iota_rep.rearrange("p (c a) -> p c a", c=C) \
                    .unsqueeze(1).broadcast_to([128, L, C, 32])
                ve.tensor_tensor(
                    masksC.rearrange("p (jq ca) -> p jq ca", jq=NQ)[:, jq0:jq0 + L]
                        .rearrange("p jq (c a) -> p jq c a", c=C),
                    cls_b, iota_b, OP.is_equal)
                # cls_perm chunk: (jq, i, q) order for the masksJ STT
                ve.tensor_copy(
                    cls_perm.rearrange("p (jq i q) -> p jq i q",
                                       jq=NQ, i=IPC)[:, jq0:jq0 + L],
                    cls16.rearrange("p (jq q i) -> p jq i q",
                                    jq=NQ, q=4)[:, jq0:jq0 + L])
                # masksJ (c-inner): ACT replicates cls over c (any-AP copy),
                # then a dense bf16 is_equal runs at 2x on DVE
                clsp_b = cls_perm[:, c0:c1].unsqueeze(2) \
                    .broadcast_to([128, 32 * L, C])
                cls_rep = cr.tile([128, 32 * L * C], BF16, name="cls_rep",
                                  tag="cls_rep")
                crv = cls_rep.rearrange("p (jj c) -> p jj c", c=C)
                act.copy(crv, clsp_b)
                iotaj_b = iota16.unsqueeze(1).broadcast_to([128, 32 * L, C])
                ve.tensor_tensor(masksJ3[:, 32 * jq0:32 * (jq0 + L), :],
                                 crv, iotaj_b, OP.is_equal)
                # prodm2K = prodmK^2 (dense 2x), then per-quad PE-layout copies
                ve.tensor_tensor(kj(prodm2K)[:, :, c0:c1], kj(prodmK)[:, :, c0:c1],
                                 kj(prodmK)[:, :, c0:c1], OP.mult)
                for jq in grp:
                    ve.tensor_copy(
                        prodm255[:, jq],
                        kj(prodm2K)[:, :, 32 * jq:32 * (jq + 1)]
                            .rearrange("p k (q i) -> p i q k", q=4))
                # histT per quad: [16=(q,k), 128L], weights=prodm2, moving=masks
                hist_ps = ph.tile([16, 128 * L], F32, name="hist_ps", tag="hist_ps")
                for jx, jq in enumerate(grp):
                    for i in range(IPC):
                        lhsT = prodm255[:, jq, i].rearrange("p q k -> p (q k)")
                        rhs = masksJ5[:, jq, i].rearrange("p q c -> p (q c)")
                        pe.matmul(hist_ps[:, 128 * jx:128 * (jx + 1)], lhsT, rhs,
                                  start=(i == 0), stop=(i == IPC - 1))
                # sqrt(hist) -> transpose -> 1/x on the narrow side
                # squad = min(1/sqrt(hist), 1e12) * blockmask
                scaleT = xp.tile([16, 128 * L], F32, name="scaleT", tag="scaleT")
                act.activation(scaleT, hist_ps, AF.Sqrt)
                scale_ps = psc.tile([128, 16 * L], F32, name="scale_ps", tag="scale_ps")
                for jx in range(L):
                    pe.transpose(scale_ps[:, 16 * jx:16 * (jx + 1)],
                                 scaleT[:, 128 * jx:128 * (jx + 1)],
                                 identf[:16, :16])
                scinv = xp.tile([128, 16 * L], F32, name="scinv", tag="scinv")
                ve.reciprocal(scinv, scale_ps)
                sq_view = squad[:, 0:128].rearrange("p (jq f) -> p jq f", f=16)
                bm_b = blockmask.unsqueeze(1).broadcast_to([128, L, 16])
                ve.scalar_tensor_tensor(
                    out=sq_view[:, jq0:jq0 + L],
                    in0=scinv.rearrange("p (l f) -> p l f", f=16),
                    scalar=1e12, in1=bm_b, op0=OP.min, op1=OP.mult)

                # per pair of quads: onehotT, G_B (partition-packed), G_A
                pairs = [grp[i:i + 2] for i in range(0, L, 2)]
                for pair in pairs:
                    ohs = []
                    for jq in pair:
                        oh_ps = poh.tile([128, NPT], BF16, name="oh_ps",
                                         tag="oh_ps")
                        for i in range(IPC):
                            lhsT = masksJ5[:, jq, i].rearrange("p q c -> p (q c)")
                            pe.transpose(oh_ps[:, 128 * i:128 * (i + 1)],
                                         lhsT, ident16)
                        oh_sb = bb.tile([128, NPT], BF16, name="oh_sb",
                                        tag="oh_sb")
                        if OHE[jq] == 'v':
                            ve.tensor_copy(oh_sb, oh_ps)
                        else:
                            act.copy(oh_sb, oh_ps)
                        ohs.append(oh_sb)
                    # 32-col squad windows keep the packed PSUM fully written
                    P = 32 * len(pair)
                    gb_sb = xp.tile([P, NPT], BF16, name="gb_sb", tag="gb_sb")
                    for h in range(2):
                        gb_ps = pgb.tile([P, 512], F32, name="gb_ps", tag="gb_ps")
                        for px, jq in enumerate(pair):
                            pe.matmul(gb_ps[32 * px:32 * px + 32, :],
                                      squad[:, 16 * jq:16 * jq + 32],
                                      ohs[px][:, 512 * h:512 * (h + 1)],
                                      start=True, stop=True)
                        act.copy(gb_sb[:, 512 * h:512 * (h + 1)], gb_ps)
                    for px, jq in enumerate(pair):
                        ga_ps = pga.tile([128, 128], BF16, name="ga_ps",
                                         tag="ga_ps")
                        p0 = 32 * px
                        for i in range(IPC):
                            pe.transpose(
                                ga_ps[:, 16 * i:16 * (i + 1)],
                                gb_sb[p0:p0 + 16, 128 * i:128 * (i + 1)],
                                ident16[p0:p0 + 16, p0:p0 + 16])
                        # gK[p, k, (jq q i)] <- ga_ps[p, (i q k)]
                        ve.tensor_copy(
                            gK.rearrange("p (k jq q i) -> p jq k q i",
                                         jq=NQ, q=4, i=IPC)[:, jq],
                            ga_ps.rearrange("p (i q k) -> p k q i", i=IPC, q=4))

                # stats over this group's columns (k-major)
                cs = slice(c0, c1)
                pgv = kj(pgK)[:, :, cs]
                ve.tensor_tensor(pgv, kj(prodmK)[:, :, cs], kj(gK)[:, :, cs],
                                 OP.mult)
                # S = sum_k pg ; msq = sum_k pg^2  (slice adds, dense 2x)
                ve.tensor_tensor(kj(pg2K)[:, :, cs], pgv, pgv, OP.mult)
                ve.tensor_tensor(ssum[:, cs].unsqueeze(1), pgv[:, 0:1],
                                 pgv[:, 1:2], OP.add)
                ve.tensor_tensor(msum[:, cs].unsqueeze(1), kj(pg2K)[:, 0:1, cs],
                                 kj(pg2K)[:, 1:2, cs], OP.add)
                for kk in (2, 3):
                    ve.tensor_tensor(ssum[:, cs].unsqueeze(1),
                                     ssum[:, cs].unsqueeze(1),
                                     pgv[:, kk:kk + 1], OP.add)
                    ve.tensor_tensor(msum[:, cs].unsqueeze(1),
                                     msum[:, cs].unsqueeze(1),
                                     kj(pg2K)[:, kk:kk + 1, cs], OP.add)
                # var*(D-1) = msq - S^2/D ; istd = 1/(sqrt(var)+eps')
                ve.scalar_tensor_tensor(out=var[:, cs], in0=ssum[:, cs],
                                        scalar=float(-1.0 / D), in1=ssum[:, cs],
                                        op0=OP.mult, op1=OP.mult)
                ve.scalar_tensor_tensor(out=var[:, cs], in0=var[:, cs], scalar=0.0,
                                        in1=msum[:, cs], op0=OP.add, op1=OP.add)
                ve.tensor_scalar(out=var[:, cs], in0=var[:, cs], scalar1=0.0,
                                 scalar2=None, op0=OP.max)
                act.activation(std[:, cs], var[:, cs], AF.Sqrt)
                act.activation(std[:, cs], std[:, cs], AF.Identity,
                               bias=bconst[:, 2:3], scale=1.0)
                ve.reciprocal_approx_fast(istd[:, cs], std[:, cs])
                ve.scalar_tensor_tensor(out=nm16[:, cs], in0=ssum[:, cs],
                                        scalar=float(-SQD1 / D), in1=istd[:, cs],
                                        op0=OP.mult, op1=OP.mult)
                # pgi = (pg * SQD1) * istd  (k-major bf16)
                istd_b = istd[:, cs].unsqueeze(1).broadcast_to([128, K, c1 - c0])
                ve.scalar_tensor_tensor(out=kj(pgiK)[:, :, cs], in0=pgv,
                                        scalar=float(SQD1), in1=istd_b,
                                        op0=OP.mult, op1=OP.mult)

            def final(grp):
                for jq in grp:
                    js = slice(32 * jq, 32 * (jq + 1))
                    X = op_.tile([128, K * C * 32], BF16, name="X", tag="X")
                    X4 = X.rearrange("p (k c a) -> p k c a", k=K, c=C)
                    mask_b = masksC4[:, jq].rearrange("p c q i -> p (c q i)") \
                        .rearrange("p (c a) -> p c a", c=C) \
                        .unsqueeze(1).broadcast_to([128, K, C, 32])
                    pgi_b = kj(pgiK)[:, :, js].unsqueeze(2) \
                        .broadcast_to([128, K, C, 32])
                    e1 = ve if S1Q[jq] == 'v' else gp
                    e1.tensor_tensor(X4, mask_b, pgi_b, OP.mult)
                    nm_b = nm16[:, js].unsqueeze(1).unsqueeze(1) \
                        .broadcast_to([128, K, C, 32])
                    e2 = ve if S2Q[jq] == 'v' else gp
                    e2.tensor_tensor(X4, X4, nm_b, OP.add)
                    sy.dma_start(out_d[jq], X)

            do_group(GROUPS[0])
            do_group(GROUPS[1])
            final(GROUPS[0])
            do_group(GROUPS[2])
            final(GROUPS[1])
            do_group(GROUPS[3])
            final(GROUPS[2])
            final(GROUPS[3])

    if not nc.is_finalized():
        nc.finalize()
    return nc


_NC = None


def _get_nc():
    global _NC
    if _NC is None:
        _NC = build_nc()
    return _NC


def _unscramble(raw: np.ndarray) -> np.ndarray:
    """[NQ, 128, K*C*32] scrambled -> [EX, NPT, D] (still source dtype)."""
    t = np.asarray(raw).reshape(NQ, 128, K, C, 4, IPC)
    # [jq, p, k, c, q, i] -> [jq, q, p, i, k, c]
    return np.transpose(t, (0, 4, 1, 5, 2, 3)).reshape(EX, NPT, D)


def kernel(x: np.ndarray) -> np.ndarray:
    from concourse.bass_utils import run_bass_kernel_spmd

    x = np.ascontiguousarray(np.asarray(x, dtype=np.float32))
    B = x.shape[0]
    n_cores = 8
    per = B // n_cores
    cf = _consts_f32()
    nc = _get_nc()
    in_maps = [
        {"x": x[i * per:(i + 1) * per], "cf": cf} for i in range(n_cores)
    ]
    res = run_bass_kernel_spmd(nc, in_maps, core_ids=list(range(n_cores)))
    return np.concatenate(
        [_unscramble(r["out"]).astype(np.float32) for r in res.results], axis=0)


if __name__ == "__main__":
    from concourse.bass_interp import CoreSim

    rng = np.random.default_rng(0)
    x = (rng.standard_normal((EX, NPT, 4)) * 2.0).astype(np.float32)
    x[..., 3] = rng.integers(0, C, size=(EX, NPT)).astype(np.float32)
    nc = build_nc()
    sim = CoreSim(nc)
    sim.tensor("x")[:] = x
    sim.tensor("cf")[:] = _consts_f32()
    sim.simulate()
    got = _unscramble(np.array(sim.tensor("out"))).astype(np.float32)

    xyz = x[..., :3]; clsf_ = x[..., 3]
    r = np.sqrt((xyz * xyz).sum(-1)); rinv = 1.0 / r
    radial = np.exp(-ETA * (np.array(RS, np.float32)[None, None] - r[..., None]) ** 2)
    env = 0.5 * np.cos(np.pi * np.minimum(r, RC) / RC) + 0.5
    sh = np.stack([np.full_like(r, Y00), C1 * xyz[..., 1] * rinv,
                   C1 * xyz[..., 2] * rinv, C1 * xyz[..., 0] * rinv], -1)
    prod = sh * radial * env[..., None]
    onehot = (clsf_[..., None] == np.arange(C, dtype=np.float32)).astype(np.float32)
    pos = (prod[..., :, None] * onehot[..., None, :]).reshape(EX, NPT, D)
    norm = np.sqrt((pos * pos).sum(1, keepdims=True))
    pos = pos / np.maximum(norm, 1e-12)
    mean_ = pos.mean(-1, keepdims=True)
    std_ = pos.std(-1, ddof=1, keepdims=True)
    want = (pos - mean_) / (std_ + 1e-6)
    err = np.abs(got - want)
    rel = np.linalg.norm((got - want).ravel()) / np.linalg.norm(want.ravel())
    print("sim absmax err:", err.max(), "rel:", rel, "ref absmax:", np.abs(want).max())
